# revision 12
# baseline (speedup 1.0000x reference)
"""Trainium2 Bass kernel for nn_DiscreteAutoregressiveFlow (sampling, forward).

Math: `inputs` is an exact one-hot [B, L, V] tensor. For a row holding token v:
  net = W[v] + b                      (exact: one-hot @ W picks a row)
  loc = one_hot(argmax(net[:V]));  scale = one_hot(argmax(net[V:]))
  one_hot_multiply -> one-hot at (scale_tok*v) % V   (zero row if scale_tok==0)
  one_hot_add      -> one-hot at (scale_tok*v + loc_tok) % V
So out[row] = one_hot(cmap[v]) with a host-precomputed 64-entry map
(sentinel >= V encodes the zero row). The straight-through softmax residuals
and FFT noise in the reference are O(1e-7) and vanish in norm relative error.

Fast path (cmap has no sentinel tokens - true for this instance; checked on
the host, with a general fallback):
  xb   = SWDGE cast-DMA in (gpsimd ring): f32 HBM -> bf16 SBUF. The convert
         rides the SDMA datapath, so no engine spends a pass on it.
  prod = xb + cmapf                  (DVE TT add, all-bf16 -> 2x mode ~600ns)
  m    = reduce_max(prod, inner V)   (DVE ~1.13us) = 1 + cmap[tok]/128 exact
  idx  = (m*128) + (64j - 128)       (DVE scalar_tensor_tensor, int16, tiny)
  out  = gpsimd.local_scatter(bf16): zero the [128, r*64] bf16 tile, write
         bf16 1.0 at idx per row-group (~1.07us/chunk, one op)
  y    = SWDGE cast-DMA out: bf16 SBUF -> f32 HBM (exact for 0.0/1.0)
The last chunk instead uses DVE is_equal + a plain HWDGE store so the final
transfer skips the ~2us SWDGE completion latency. Rate math: input DMAs land
every ~1.35us; DVE needs ~1.83us/chunk and Pool ~1.7us/chunk, so the output
stream stays within ~1 chunk of the input stream and the DMA engines never
starve for long. All f32/bf16 values involved are exact (c <= 127 with 2^-7
scaling), so every comparison/convert is exact.

General path (sentinel present): ACT bf16-cast copies + DVE is_equal
one-hot generation against the broadcast max (sentinel max matches no iota
entry -> zero row).

Sharding: pure data parallel over B*L rows, 8 cores, no collectives.
"""

import numpy as np
import ml_dtypes

V = 64
P = 128
N_CORES = 8
B, L = 16, 8192
ROWS = B * L                      # 131072
ROWS_PER_CORE = ROWS // N_CORES   # 16384
SENTINEL = 100.0
EPS = 1.0 / 128.0

# rows per partition per chunk; chunk = [128, R*64] f32 = R*32KB
R = 16

_CACHE = {}


def _build_nc(rows_per_core: int, r: int, scatter: bool):
    import concourse.bacc as bacc
    import concourse.mybir as mybir
    from concourse.bass import broadcast_tensor_aps
    from concourse.tile import TileContext

    f32 = mybir.dt.float32
    bf16 = mybir.dt.bfloat16
    i16 = mybir.dt.int16
    fd = r * V
    chunk_rows = P * r
    n_chunks = rows_per_core // chunk_rows
    assert rows_per_core % chunk_rows == 0
    assert r == 16

    # Bacc (not raw Bass): its compile() runs generate_event_semaphores(),
    # which legalizes multi-wait instructions for TRN2 (1 wait per instr).
    nc = bacc.Bacc("TRN2", target_bir_lowering=False, name="daf_onehot")
    x = nc.dram_tensor("x", [rows_per_core, V], f32, kind="ExternalInput")
    cmapf = nc.dram_tensor("cmapf", [P, fd], bf16, kind="ExternalInput")
    cmap = nc.dram_tensor("cmap", [P, V], f32, kind="ExternalInput")
    iota = nc.dram_tensor("iota", [P, V], f32, kind="ExternalInput")
    off = nc.dram_tensor("off", [P, r], f32, kind="ExternalInput")
    ones = nc.dram_tensor("ones", [P, r], bf16, kind="ExternalInput")
    y = nc.dram_tensor("y", [rows_per_core, V], f32, kind="ExternalOutput")

    xv = x.rearrange("(c p r) v -> c p (r v)", p=P, r=r)
    yv = y.rearrange("(c p r) v -> c p (r v)", p=P, r=r)

    with TileContext(nc) as tc:
        with (
            tc.tile_pool(name="const", bufs=1) as constp,
            tc.tile_pool(name="io", bufs=n_chunks) as iop,
            tc.tile_pool(name="work", bufs=n_chunks) as workp,
        ):
            if scatter:
                _build_scatter_body(
                    nc, mybir, broadcast_tensor_aps, constp, iop, workp,
                    xv, yv, cmapf, iota, off, ones,
                    f32, bf16, i16, fd, r, n_chunks,
                )
            else:
                _build_general_body(
                    nc, mybir, broadcast_tensor_aps, constp, iop, workp,
                    xv, yv, cmap, iota, f32, bf16, fd, r, n_chunks,
                )

    # Bacc.finalize runs compile(): wait-splitting (generate_event_semaphores),
    # register allocation, nop fusion. run_bass_via_pjrt serializes nc.m as-is,
    # so this must happen here.
    nc.finalize()
    return nc


def _build_scatter_body(nc, mybir, broadcast_tensor_aps, constp, iop, workp,
                        xv, yv, cmapf, iota, off, ones,
                        f32, bf16, i16, fd, r, n_chunks):
    P_ = P
    # Constants on the scalar (ACT) HWDGE ring; cmapf first since the first
    # DVE add waits on it.
    cmapf_t = constp.tile([P_, fd], bf16, tag="cmapf_t")
    nc.scalar.dma_start(cmapf_t[:], cmapf[:])
    off_t = constp.tile([P_, r], f32, tag="off_t")
    iota_st = constp.tile([P_, V], f32, tag="iota_st")
    nc.scalar.dma_start(off_t[:], off[:])
    nc.scalar.dma_start(iota_st[:], iota[:])
    iota_1 = iota_st[:].rearrange("p (o v) -> p o v", o=1)
    # The scatter's 1.0-data comes from a gpsimd memset: same-engine
    # dependency, so the first scatter never waits on a cross-ring DMA sem.
    ones_t = constp.tile([P_, r], bf16, tag="ones_t")
    nc.gpsimd.memset(ones_t[:], 1.0)

    # All input cast-DMAs issued first: the Q7 descriptor generations run
    # back-to-back so the input stream flows at line rate.
    xbs = []
    for ci in range(n_chunks):
        xb = iop.tile([P_, fd], bf16, tag="x")
        nc.gpsimd.dma_start(xb[:], xv[ci])
        xbs.append(xb)

    for ci in range(n_chunks):
        prod = workp.tile([P_, fd], bf16, tag="prod")
        nc.vector.tensor_tensor(
            prod[:], xbs[ci][:], cmapf_t[:], op=mybir.AluOpType.add
        )
        p3 = prod[:].rearrange("p (r v) -> p r v", v=V)

        c_t = workp.tile([P_, r], f32, tag="c")
        nc.vector.tensor_reduce(
            c_t[:], p3, axis=mybir.AxisListType.X, op=mybir.AluOpType.max
        )

        if ci < n_chunks - 2:
            # idx = cmap[tok] + 64*j = (m*128) + (64j - 128), int16 on the
            # DVE write; exact integers in [0, fd).
            idx16 = workp.tile([P_, r], i16, tag="idx16")
            nc.vector.scalar_tensor_tensor(
                idx16[:], c_t[:], 128.0, off_t[:],
                op0=mybir.AluOpType.mult, op1=mybir.AluOpType.add,
            )
            out_t = iop.tile([P_, fd], bf16, tag="out")
            nc.gpsimd.local_scatter(
                out_t[:], ones_t[:], idx16[:],
                channels=P_, num_elems=fd, num_idxs=r,
            )
            # SWDGE cast store: bf16 0.0/1.0 -> f32, exact.
            nc.gpsimd.dma_start(yv[ci], out_t[:])
        else:
            # Last two chunks: DVE is_equal + plain HWDGE store, so the
            # final transfers skip the SWDGE completion latency.
            out_f = iop.tile([P_, fd], f32, tag="outf")
            o3 = out_f[:].rearrange("p (r v) -> p r v", v=V)
            c3 = c_t[:].rearrange("p (r one) -> p r one", one=1)
            c3_b, _ = broadcast_tensor_aps(c3, o3)
            io_b, _ = broadcast_tensor_aps(iota_1, o3)
            nc.vector.tensor_tensor(o3, io_b, c3_b, op=mybir.AluOpType.is_equal)
            nc.sync.dma_start(yv[ci], out_f[:])


def _build_general_body(nc, mybir, broadcast_tensor_aps, constp, iop, workp,
                        xv, yv, cmap, iota, f32, bf16, fd, r, n_chunks):
    P_ = P
    cmap_st = constp.tile([P_, V], f32, tag="cmap_st")
    iota_st = constp.tile([P_, V], f32, tag="iota_st")
    nc.scalar.dma_start(cmap_st[:], cmap[:])
    nc.scalar.dma_start(iota_st[:], iota[:])
    iota_1 = iota_st[:].rearrange("p (o v) -> p o v", o=1)

    xts = []
    for ci in range(n_chunks):
        xt = iop.tile([P_, fd], f32, tag="x")
        nc.sync.dma_start(xt[:], xv[ci])
        xts.append(xt)

    cmap_1 = cmap_st[:].rearrange("p (o v) -> p o v", o=1)
    cmapf = constp.tile([P_, fd], bf16, tag="cmapf")
    cf3 = cmapf[:].rearrange("p (r v) -> p r v", v=V)
    cm_b, _ = broadcast_tensor_aps(cmap_1, cf3)
    nc.scalar.copy(cf3, cm_b)

    xbs = []
    for ci in range(n_chunks):
        xb = workp.tile([P_, fd], bf16, tag="xb")
        nc.scalar.copy(xb[:], xts[ci][:])
        xbs.append(xb)

    for ci in range(n_chunks):
        prod = workp.tile([P_, fd], bf16, tag="prod")
        nc.vector.tensor_tensor(
            prod[:], xbs[ci][:], cmapf[:], op=mybir.AluOpType.add
        )
        p3 = prod[:].rearrange("p (r v) -> p r v", v=V)

        c_t = workp.tile([P_, r], f32, tag="c")
        nc.vector.tensor_reduce(
            c_t[:], p3, axis=mybir.AxisListType.X, op=mybir.AluOpType.max
        )

        out_t = iop.tile([P_, fd], f32, tag="out")
        o3 = out_t[:].rearrange("p (r v) -> p r v", v=V)
        c3 = c_t[:].rearrange("p (r one) -> p r one", one=1)
        c3_b, _ = broadcast_tensor_aps(c3, o3)
        io_b, _ = broadcast_tensor_aps(iota_1, o3)
        nc.vector.tensor_tensor(o3, io_b, c3_b, op=mybir.AluOpType.is_equal)
        nc.sync.dma_start(yv[ci], out_t[:])


def _get_nc(rows_per_core=ROWS_PER_CORE, r=R, scatter=False):
    key = (rows_per_core, r, scatter)
    if key not in _CACHE:
        _CACHE[key] = _build_nc(rows_per_core, r, scatter)
    return _CACHE[key]


def _host_cmap(W: np.ndarray, b: np.ndarray) -> np.ndarray:
    """64-entry map token -> output one-hot index (or sentinel for zero row)."""
    net = W.astype(np.float32) + b.astype(np.float32)[None, :]   # [V, 2V]
    loc_tok = np.argmax(net[:, :V], axis=1)                      # [V]
    scale_tok = np.argmax(net[:, V:], axis=1)                    # [V]
    t = (scale_tok * np.arange(V, dtype=np.int64) + loc_tok) % V
    return np.where(scale_tok == 0, SENTINEL, t.astype(np.float64)).astype(
        np.float32
    )


def _host_tables(W: np.ndarray, b: np.ndarray):
    cmap_eps = _host_cmap(W, b) * np.float32(EPS)                  # exact f32
    iota_eps = 1.0 + np.arange(V, dtype=np.float32) * np.float32(EPS)
    cmap_t = np.tile(cmap_eps.astype(np.float32)[None, :], (P, 1))
    iota_t = np.tile(iota_eps.astype(np.float32)[None, :], (P, 1))
    return cmap_t, iota_t


def _in_maps(inputs: np.ndarray, W: np.ndarray, b: np.ndarray):
    x = np.ascontiguousarray(inputs.astype(np.float32, copy=False).reshape(ROWS, V))
    cmap_t, iota_t = _host_tables(W, b)
    cmapf = np.tile(cmap_t[:, :V].astype(ml_dtypes.bfloat16), (1, R))
    off = np.tile(
        (64.0 * np.arange(R) - 128.0).astype(np.float32)[None, :], (P, 1)
    )
    ones = np.ones((P, R), dtype=ml_dtypes.bfloat16)
    return [
        {
            "x": x[c * ROWS_PER_CORE : (c + 1) * ROWS_PER_CORE],
            "cmapf": cmapf,
            "cmap": cmap_t,
            "iota": iota_t,
            "off": off,
            "ones": ones,
        }
        for c in range(N_CORES)
    ]


def _use_scatter(W: np.ndarray, b: np.ndarray) -> bool:
    """Fast path is exact iff no token maps to the zero row (no sentinel)."""
    return not np.any(_host_cmap(W, b) >= np.float32(V))


def kernel(inputs: np.ndarray, W: np.ndarray, b: np.ndarray) -> np.ndarray:
    from concourse import bass_utils

    nc = _get_nc(scatter=_use_scatter(W, b))
    in_maps = _in_maps(inputs, W, b)
    res = bass_utils.run_bass_kernel_spmd(nc, in_maps, core_ids=list(range(N_CORES)))
    y = np.concatenate([r["y"] for r in res.results], axis=0)
    return y.reshape(inputs.shape).astype(inputs.dtype, copy=False)


# revision 14
# speedup vs baseline: 1.1558x; 1.1558x over previous
"""Trainium2 Bass kernel for nn_DiscreteAutoregressiveFlow (sampling, forward).

Math: `inputs` is an exact one-hot [B, L, V] tensor. For a row holding token v:
  net = W[v] + b                      (exact: one-hot @ W picks a row)
  loc = one_hot(argmax(net[:V]));  scale = one_hot(argmax(net[V:]))
  one_hot_multiply -> one-hot at (scale_tok*v) % V   (zero row if scale_tok==0)
  one_hot_add      -> one-hot at (scale_tok*v + loc_tok) % V
So out[row] = one_hot(cmap[v]) with a host-precomputed 64-entry map
(sentinel >= V encodes the zero row). The straight-through softmax residuals
and FFT noise in the reference are O(1e-7) and vanish in norm relative error.

Fast path (cmap has no sentinel tokens - true for this instance; checked on
the host, with a general fallback):
  xb   = SWDGE cast-DMA in (gpsimd ring): f32 HBM -> bf16 SBUF. The convert
         rides the SDMA datapath, so no engine spends a pass on it.
  prod = xb + cmapf                  (DVE TT add, all-bf16 -> 2x mode ~600ns)
  m    = reduce_max(prod, inner V)   (DVE ~1.13us) = 1 + cmap[tok]/128 exact
  idx  = (m*128) + (64j - 128)       (DVE scalar_tensor_tensor, int16, tiny)
  out  = gpsimd.local_scatter(bf16): zero the [128, r*64] bf16 tile, write
         bf16 1.0 at idx per row-group (~1.07us/chunk, one op)
  y    = SWDGE cast-DMA out: bf16 SBUF -> f32 HBM (exact for 0.0/1.0)
The last chunk instead uses DVE is_equal + a plain HWDGE store so the final
transfer skips the ~2us SWDGE completion latency. Rate math: input DMAs land
every ~1.35us; DVE needs ~1.83us/chunk and Pool ~1.7us/chunk, so the output
stream stays within ~1 chunk of the input stream and the DMA engines never
starve for long. All f32/bf16 values involved are exact (c <= 127 with 2^-7
scaling), so every comparison/convert is exact.

General path (sentinel present): ACT bf16-cast copies + DVE is_equal
one-hot generation against the broadcast max (sentinel max matches no iota
entry -> zero row).

Sharding: pure data parallel over B*L rows, 8 cores, no collectives.
"""

import numpy as np
import ml_dtypes

V = 64
P = 128
N_CORES = 8
B, L = 16, 8192
ROWS = B * L                      # 131072
ROWS_PER_CORE = ROWS // N_CORES   # 16384
SENTINEL = 100.0
EPS = 1.0 / 128.0

# rows per partition per chunk; chunk = [128, R*64] f32 = R*32KB
R = 16

_CACHE = {}


def _build_nc(rows_per_core: int, r: int, scatter: bool):
    import concourse.bacc as bacc
    import concourse.mybir as mybir
    from concourse.bass import broadcast_tensor_aps
    from concourse.tile import TileContext

    f32 = mybir.dt.float32
    bf16 = mybir.dt.bfloat16
    i16 = mybir.dt.int16
    fd = r * V
    chunk_rows = P * r
    n_chunks = rows_per_core // chunk_rows
    assert rows_per_core % chunk_rows == 0
    assert r == 16

    # Bacc (not raw Bass): its compile() runs generate_event_semaphores(),
    # which legalizes multi-wait instructions for TRN2 (1 wait per instr).
    nc = bacc.Bacc("TRN2", target_bir_lowering=False, name="daf_onehot")
    x = nc.dram_tensor("x", [rows_per_core, V], f32, kind="ExternalInput")
    cmapf = nc.dram_tensor("cmapf", [P, fd], bf16, kind="ExternalInput")
    cmap = nc.dram_tensor("cmap", [P, V], f32, kind="ExternalInput")
    iota = nc.dram_tensor("iota", [P, V], f32, kind="ExternalInput")
    off = nc.dram_tensor("off", [P, r], f32, kind="ExternalInput")
    ones = nc.dram_tensor("ones", [P, r], bf16, kind="ExternalInput")
    y = nc.dram_tensor("y", [rows_per_core, V], f32, kind="ExternalOutput")

    xv = x.rearrange("(c p r) v -> c p (r v)", p=P, r=r)
    yv = y.rearrange("(c p r) v -> c p (r v)", p=P, r=r)

    with TileContext(nc) as tc:
        with (
            tc.tile_pool(name="const", bufs=1) as constp,
            tc.tile_pool(name="io", bufs=n_chunks) as iop,
            tc.tile_pool(name="work", bufs=n_chunks) as workp,
        ):
            if scatter:
                _build_scatter_body(
                    nc, mybir, broadcast_tensor_aps, constp, iop, workp,
                    xv, yv, cmapf, iota, off, ones,
                    f32, bf16, i16, fd, r, n_chunks,
                )
            else:
                _build_general_body(
                    nc, mybir, broadcast_tensor_aps, constp, iop, workp,
                    xv, yv, cmap, iota, f32, bf16, fd, r, n_chunks,
                )

    # Bacc.finalize runs compile(): wait-splitting (generate_event_semaphores),
    # register allocation, nop fusion. run_bass_via_pjrt serializes nc.m as-is,
    # so this must happen here.
    nc.finalize()
    return nc


def _build_scatter_body(nc, mybir, broadcast_tensor_aps, constp, iop, workp,
                        xv, yv, cmapf, iota, off, ones,
                        f32, bf16, i16, fd, r, n_chunks):
    P_ = P
    # Constants on the scalar (ACT) HWDGE ring; cmapf first since the first
    # DVE add waits on it.
    cmapf_t = constp.tile([P_, fd], bf16, tag="cmapf_t")
    nc.scalar.dma_start(cmapf_t[:], cmapf[:])
    off_t = constp.tile([P_, r], f32, tag="off_t")
    iota_st = constp.tile([P_, V], f32, tag="iota_st")
    nc.scalar.dma_start(off_t[:], off[:])
    nc.scalar.dma_start(iota_st[:], iota[:])
    iota_1 = iota_st[:].rearrange("p (o v) -> p o v", o=1)
    # The scatter's 1.0-data comes from a gpsimd memset: same-engine
    # dependency, so the first scatter never waits on a cross-ring DMA sem.
    ones_t = constp.tile([P_, r], bf16, tag="ones_t")
    nc.gpsimd.memset(ones_t[:], 1.0)

    # Dummy scatter BEFORE the input DMAs: forces the Q7 library load while
    # the DMA queues are still empty. Otherwise the library's own DMA queues
    # behind the eight 512KB input transfers and the first real scatter
    # stalls ~3us waiting for it. The dummy's output is scratch (duplicate
    # zero indices just overwrite the same cell).
    dummy_o = constp.tile([P_, 2], bf16, tag="dummy_o")
    dummy_i = constp.tile([P_, 2], i16, tag="dummy_i")
    nc.gpsimd.memset(dummy_i[:], 0)
    nc.gpsimd.local_scatter(
        dummy_o[:], ones_t[:].rearrange("p (a b) -> a p b", a=r // 2)[0],
        dummy_i[:], channels=P_, num_elems=2, num_idxs=2,
    )

    # All input cast-DMAs issued first: the Q7 descriptor generations run
    # back-to-back so the input stream flows at line rate.
    xbs = []
    for ci in range(n_chunks):
        xb = iop.tile([P_, fd], bf16, tag="x")
        nc.gpsimd.dma_start(xb[:], xv[ci])
        xbs.append(xb)

    for ci in range(n_chunks):
        prod = workp.tile([P_, fd], bf16, tag="prod")
        nc.vector.tensor_tensor(
            prod[:], xbs[ci][:], cmapf_t[:], op=mybir.AluOpType.add
        )
        p3 = prod[:].rearrange("p (r v) -> p r v", v=V)

        c_t = workp.tile([P_, r], f32, tag="c")
        nc.vector.tensor_reduce(
            c_t[:], p3, axis=mybir.AxisListType.X, op=mybir.AluOpType.max
        )

        if ci < n_chunks - 2:
            # idx = cmap[tok] + 64*j = (m*128) + (64j - 128), int16 on the
            # DVE write; exact integers in [0, fd).
            idx16 = workp.tile([P_, r], i16, tag="idx16")
            nc.vector.scalar_tensor_tensor(
                idx16[:], c_t[:], 128.0, off_t[:],
                op0=mybir.AluOpType.mult, op1=mybir.AluOpType.add,
            )
            out_t = iop.tile([P_, fd], bf16, tag="out")
            nc.gpsimd.local_scatter(
                out_t[:], ones_t[:], idx16[:],
                channels=P_, num_elems=fd, num_idxs=r,
            )
            # SWDGE cast store: bf16 0.0/1.0 -> f32, exact.
            nc.gpsimd.dma_start(yv[ci], out_t[:])
        else:
            # Last two chunks: DVE is_equal + plain HWDGE store, so the
            # final transfers skip the SWDGE completion latency.
            out_f = iop.tile([P_, fd], f32, tag="outf")
            o3 = out_f[:].rearrange("p (r v) -> p r v", v=V)
            c3 = c_t[:].rearrange("p (r one) -> p r one", one=1)
            c3_b, _ = broadcast_tensor_aps(c3, o3)
            io_b, _ = broadcast_tensor_aps(iota_1, o3)
            nc.vector.tensor_tensor(o3, io_b, c3_b, op=mybir.AluOpType.is_equal)
            nc.sync.dma_start(yv[ci], out_f[:])


def _build_general_body(nc, mybir, broadcast_tensor_aps, constp, iop, workp,
                        xv, yv, cmap, iota, f32, bf16, fd, r, n_chunks):
    P_ = P
    cmap_st = constp.tile([P_, V], f32, tag="cmap_st")
    iota_st = constp.tile([P_, V], f32, tag="iota_st")
    nc.scalar.dma_start(cmap_st[:], cmap[:])
    nc.scalar.dma_start(iota_st[:], iota[:])
    iota_1 = iota_st[:].rearrange("p (o v) -> p o v", o=1)

    xts = []
    for ci in range(n_chunks):
        xt = iop.tile([P_, fd], f32, tag="x")
        nc.sync.dma_start(xt[:], xv[ci])
        xts.append(xt)

    cmap_1 = cmap_st[:].rearrange("p (o v) -> p o v", o=1)
    cmapf = constp.tile([P_, fd], bf16, tag="cmapf")
    cf3 = cmapf[:].rearrange("p (r v) -> p r v", v=V)
    cm_b, _ = broadcast_tensor_aps(cmap_1, cf3)
    nc.scalar.copy(cf3, cm_b)

    xbs = []
    for ci in range(n_chunks):
        xb = workp.tile([P_, fd], bf16, tag="xb")
        nc.scalar.copy(xb[:], xts[ci][:])
        xbs.append(xb)

    for ci in range(n_chunks):
        prod = workp.tile([P_, fd], bf16, tag="prod")
        nc.vector.tensor_tensor(
            prod[:], xbs[ci][:], cmapf[:], op=mybir.AluOpType.add
        )
        p3 = prod[:].rearrange("p (r v) -> p r v", v=V)

        c_t = workp.tile([P_, r], f32, tag="c")
        nc.vector.tensor_reduce(
            c_t[:], p3, axis=mybir.AxisListType.X, op=mybir.AluOpType.max
        )

        out_t = iop.tile([P_, fd], f32, tag="out")
        o3 = out_t[:].rearrange("p (r v) -> p r v", v=V)
        c3 = c_t[:].rearrange("p (r one) -> p r one", one=1)
        c3_b, _ = broadcast_tensor_aps(c3, o3)
        io_b, _ = broadcast_tensor_aps(iota_1, o3)
        nc.vector.tensor_tensor(o3, io_b, c3_b, op=mybir.AluOpType.is_equal)
        nc.sync.dma_start(yv[ci], out_t[:])


def _get_nc(rows_per_core=ROWS_PER_CORE, r=R, scatter=False):
    key = (rows_per_core, r, scatter)
    if key not in _CACHE:
        _CACHE[key] = _build_nc(rows_per_core, r, scatter)
    return _CACHE[key]


def _host_cmap(W: np.ndarray, b: np.ndarray) -> np.ndarray:
    """64-entry map token -> output one-hot index (or sentinel for zero row)."""
    net = W.astype(np.float32) + b.astype(np.float32)[None, :]   # [V, 2V]
    loc_tok = np.argmax(net[:, :V], axis=1)                      # [V]
    scale_tok = np.argmax(net[:, V:], axis=1)                    # [V]
    t = (scale_tok * np.arange(V, dtype=np.int64) + loc_tok) % V
    return np.where(scale_tok == 0, SENTINEL, t.astype(np.float64)).astype(
        np.float32
    )


def _host_tables(W: np.ndarray, b: np.ndarray):
    cmap_eps = _host_cmap(W, b) * np.float32(EPS)                  # exact f32
    iota_eps = 1.0 + np.arange(V, dtype=np.float32) * np.float32(EPS)
    cmap_t = np.tile(cmap_eps.astype(np.float32)[None, :], (P, 1))
    iota_t = np.tile(iota_eps.astype(np.float32)[None, :], (P, 1))
    return cmap_t, iota_t


def _in_maps(inputs: np.ndarray, W: np.ndarray, b: np.ndarray):
    x = np.ascontiguousarray(inputs.astype(np.float32, copy=False).reshape(ROWS, V))
    cmap_t, iota_t = _host_tables(W, b)
    cmapf = np.tile(cmap_t[:, :V].astype(ml_dtypes.bfloat16), (1, R))
    off = np.tile(
        (64.0 * np.arange(R) - 128.0).astype(np.float32)[None, :], (P, 1)
    )
    ones = np.ones((P, R), dtype=ml_dtypes.bfloat16)
    return [
        {
            "x": x[c * ROWS_PER_CORE : (c + 1) * ROWS_PER_CORE],
            "cmapf": cmapf,
            "cmap": cmap_t,
            "iota": iota_t,
            "off": off,
            "ones": ones,
        }
        for c in range(N_CORES)
    ]


def _use_scatter(W: np.ndarray, b: np.ndarray) -> bool:
    """Fast path is exact iff no token maps to the zero row (no sentinel)."""
    return not np.any(_host_cmap(W, b) >= np.float32(V))


def kernel(inputs: np.ndarray, W: np.ndarray, b: np.ndarray) -> np.ndarray:
    from concourse import bass_utils

    nc = _get_nc(scatter=_use_scatter(W, b))
    in_maps = _in_maps(inputs, W, b)
    res = bass_utils.run_bass_kernel_spmd(nc, in_maps, core_ids=list(range(N_CORES)))
    y = np.concatenate([r["y"] for r in res.results], axis=0)
    return y.reshape(inputs.shape).astype(inputs.dtype, copy=False)
